# revision 26
# baseline (speedup 1.0000x reference)
"""Augmented Chamfer loss on 8 Trainium2 NeuronCores.

reference math (per batch b):
    P[i, j] = ||gts[b, i] - preds[b, j]||^2           (4096 x 4096)
    loss_1  = mean over (b, j) of min_i P             (col-min)
    loss_2  = mean over (b, i) of min_j P             (row-min)
    out     = max(loss_1, loss_2)

Sharding: data-parallel over batch, one batch element per core (B=8).

Per-core device plan:
  - PE computes P directly via an augmented contraction
      lhsT rows = [-2*gx, -2*gy, -2*gz, 1, gg]   (i along free dim)
      rhs  rows = [ px,    py,    pz,   pp, 1]   (j along free dim)
    in fp16 hi/lo arithmetic: each operand is split x = hi + lo and the
    pieces are stacked along K ([A_hi; A_lo; A_hi] x [B_hi; B_hi; B_lo]),
    so a single K=15 fp16 matmul (1 col/cycle -- 4x faster than fp32)
    yields a near-fp32-accurate P tile in PSUM.  4 matmuls of N=512 in
    distinct PE row-groups fill a [128 i x 2048 j] PSUM group.
  - ACT (scalar engine) drains each PSUM group to SBUF as fp16 (the
    unavoidable 1x-rate first touch).
  - DVE (vector engine): running elementwise fp16 min across i-tiles
    (column mins, 2x mode) + one fused custom-DVE op per i-tile
    (elementwise min + free-dim min-reduce) giving the row min over all
    4096 j.  (The stock TENSOR_TENSOR_REDUCE instruction crashes at
    runtime in this environment; the custom-DVE table mechanism works.)
  - Host: gathers tiny per-core partials ([128,32] row mins, [128,4096]
    column-min partials), finishes means + max.

Measured on the 8-core axon trn2 pod: final scalar relative error 3.3e-5
vs the jax fp32 reference; device time ~155 us (wall-clock slope of
device-side repetitions; the per-dispatch axon overhead is ~0.15-0.4 s).
"""

import os

import numpy as np

B = 8
N = 4096
N_CORES = 8
TILE_P = 128
JCHUNK = 2048
N_ITILES = N // TILE_P  # 32
N_JCH = N // JCHUNK  # 2

# Device-side repetition of the whole compute loop (timing experiments only).
REPS = int(os.environ.get("CHAMFER_REPS", "1"))
# Pipeline stage selector for timing bisection: mm | act | col | full
STAGE = os.environ.get("CHAMFER_STAGE", "full")
# PE operand dtype for the hi/lo split matmuls: float16 (more precise) or
# bfloat16 (documented-fast fallback).
HILO_DTYPE = os.environ.get("CHAMFER_HILO_DTYPE", "float16")

_STATE: dict = {}


def _register_min_op():
    """Custom DVE op: out = min(in0, in1); accum_out = min(s0, min_k out[k]).

    The stock TENSOR_TENSOR_REDUCE instruction crashes at runtime in this
    environment, so the same fusion is expressed through the (production)
    custom-DVE table mechanism instead: one DVE pass gives both the
    elementwise min (column-min premin) and the free-dim min (row min).
    """
    import concourse.dve_ops as dve_ops
    from concourse.dve_ops import DveOp
    from concourse.dve_spec import Spec, Src0, Src1, C0, minn, lower
    from concourse.dve_uop import DveOpSpec

    NAME = "TT_MIN_MIN_ANT"
    if NAME in dve_ops._SUB_OPCODE_FOR_NAME:
        return next(op for op in dve_ops.OPS if op.name == NAME)

    def _ref(in0, in1, c0, c1, c2):
        body = np.fmin(np.asarray(in0, np.float32), np.asarray(in1, np.float32))
        b2 = body.reshape(body.shape[0], -1)
        acc = np.fmin(np.fmin.reduce(b2, axis=-1, keepdims=True), c0)
        return body, acc

    spec = Spec(body=minn(Src0, Src1), accum=minn, accum_init=C0, reference=_ref)
    row = dve_ops._CUSTOM_DVE_ROW_BASE + len(dve_ops.OPS)
    assert row < 0x20, "custom-DVE row field overflow"
    shas = {}
    for ver in ("v3", "v4"):
        uops = lower(spec, ver=ver)
        shas[ver] = DveOpSpec(name=NAME, opcode=row, uops=uops, rd1_en=True).sha(ver)
    op = DveOp(NAME, spec, subdim=False, uops_sha=shas)
    dve_ops.OPS.append(op)
    dve_ops._SUB_OPCODE_FOR_NAME[NAME] = row
    dve_ops.CUSTOM_DVE_SPECS[NAME] = spec
    return op


def _build_nc():
    import concourse.bacc as bacc
    import concourse.tile as tile
    from concourse import mybir

    f16 = mybir.dt.float16
    f32 = mybir.dt.float32
    mm_dt = getattr(mybir.dt, HILO_DTYPE)
    amin = mybir.AluOpType.min
    min_op = _register_min_op()

    nc = bacc.Bacc("TRN2", target_bir_lowering=False, debug=False)
    # lr rows 0-14: lhsT = [A_hi; A_lo; A_hi], rows 15-29: rhs = [B_hi; B_hi; B_lo].
    # One K=15 matmul then computes A_hi*B_hi + A_lo*B_hi + A_hi*B_lo — the
    # full fp16 hi/lo product (lo*lo dropped, ~2^-22 relative) at the cost of
    # a K=5 matmul (fp16 streams 1 col/cycle vs 4 for fp32).
    lr = nc.dram_tensor("lr", [30, N], mm_dt, kind="ExternalInput")
    rowmins = nc.dram_tensor("rowmins", [TILE_P, N_ITILES], f32, kind="ExternalOutput")
    colmins = nc.dram_tensor("colmins", [TILE_P, N], f16, kind="ExternalOutput")

    with tile.TileContext(nc) as tc:
        with (
            tc.tile_pool(name="w", bufs=1) as wpool,
            tc.tile_pool(name="psum", bufs=2, space="PSUM") as ppool,
            tc.tile_pool(name="f16", bufs=6) as fpool,
            tc.tile_pool(name="acc", bufs=1) as apool,
        ):
            lhsTq = wpool.tile([TILE_P, N], mm_dt, tag="lhsT")
            rhsq = wpool.tile([TILE_P, N], mm_dt, tag="rhs")
            # Replicate the stacked hi/lo operand rows into all four PE
            # row-groups (each matmul's operands must start at the group base).
            for r in range(4):
                nc.sync.dma_start(lhsTq[32 * r : 32 * r + 15, :], lr.ap()[0:15, :])
                nc.sync.dma_start(rhsq[32 * r : 32 * r + 15, :], lr.ap()[15:30, :])

            colacc = apool.tile([TILE_P, N], f16, tag="colacc")
            rowacc = apool.tile([TILE_P, N_ITILES], f32, tag="rowacc")
            if STAGE != "full":
                nc.gpsimd.memset(rowacc[:], 0.0)
                if STAGE in ("mm", "act"):
                    nc.gpsimd.memset(colacc[:], 0.0)

            for t in [ti for _ in range(REPS) for ti in range(N_ITILES)]:
                ft = fpool.tile([TILE_P, N], f16, tag="ft")
                for h in range(N_JCH):
                    ps = ppool.tile([TILE_P, JCHUNK], f32, tag="ps")
                    isl = slice(t * 128, (t + 1) * 128)
                    for r in range(4):
                        j0 = h * JCHUNK + r * 512
                        nc.tensor.matmul(
                            ps[:, r * 512 : (r + 1) * 512],
                            lhsTq[32 * r : 32 * r + 15, isl],
                            rhsq[32 * r : 32 * r + 15, j0 : j0 + 512],
                            start=True,
                            stop=True,
                            tile_position=(32 * r, 0),
                        )
                    if STAGE != "mm":
                        nc.scalar.copy(ft[:, h * JCHUNK : (h + 1) * JCHUNK], ps[:])
                if STAGE in ("col", "full"):
                    # Column mins: one running elementwise fp16 min (2x mode).
                    if t == 0:
                        nc.vector.tensor_copy(colacc[:], ft[:])
                    else:
                        nc.vector.tensor_tensor(colacc[:], ft[:], colacc[:], op=amin)
                if STAGE == "full":
                    # Row-min: one fused custom-DVE pass over the two halves —
                    # elementwise min to pm plus min-reduce into rowacc.
                    pm = fpool.tile([TILE_P, JCHUNK], f16, tag="pm")
                    nc.vector._custom_dve(
                        min_op,
                        out=pm[:],
                        in0=ft[:, 0:JCHUNK],
                        in1=ft[:, JCHUNK:N],
                        s0=60000.0,
                        accum_out=rowacc[:, t : t + 1],
                    )

            nc.sync.dma_start(rowmins.ap(), rowacc[:])
            nc.sync.dma_start(colmins.ap(), colacc[:])

    nc.compile()
    return nc


def _get_nc():
    if "nc" not in _STATE:
        _STATE["nc"] = _build_nc()
    return _STATE["nc"]


def _np_hilo_dtype():
    if HILO_DTYPE == "float16":
        return np.float16
    import ml_dtypes

    return ml_dtypes.bfloat16


def _split_hi_lo(x: np.ndarray):
    dt = _np_hilo_dtype()
    hi = x.astype(dt)
    lo = (x - hi.astype(np.float32)).astype(dt)
    return hi, lo


def _prep_in_maps(preds: np.ndarray, gts: np.ndarray) -> list[dict]:
    preds = np.asarray(preds, dtype=np.float32)
    gts = np.asarray(gts, dtype=np.float32)
    in_maps = []
    for b in range(B):
        g = gts[b]
        p = preds[b]
        a = np.empty((5, N), np.float32)
        a[0:3] = -2.0 * g.T
        a[3] = 1.0
        a[4] = (g * g).sum(-1)
        bb = np.empty((5, N), np.float32)
        bb[0:3] = p.T
        bb[3] = (p * p).sum(-1)
        bb[4] = 1.0
        a_hi, a_lo = _split_hi_lo(a)
        b_hi, b_lo = _split_hi_lo(bb)
        lrb = np.concatenate([a_hi, a_lo, a_hi, b_hi, b_hi, b_lo], axis=0)
        assert lrb.shape == (30, N) and lrb.dtype == _np_hilo_dtype()
        in_maps.append({"lr": lrb})
    return in_maps


def _finish(results: list[dict]) -> np.ndarray:
    rowmin_all = np.concatenate(
        [results[b]["rowmins"].reshape(-1) for b in range(B)]
    )
    colmin_all = np.concatenate(
        [results[b]["colmins"].astype(np.float32).min(axis=0) for b in range(B)]
    )
    loss_2 = rowmin_all.mean(dtype=np.float32)
    loss_1 = colmin_all.mean(dtype=np.float32)
    return np.asarray(np.maximum(loss_1, loss_2), dtype=np.float32)


def _get_runner():
    """Build + compile + jit once; return a callable in_maps -> results.

    Mirrors concourse.bass2jax.run_bass_via_pjrt's multi-core path but
    caches the jitted executable so repeat kernel() calls skip retracing.
    """
    if "runner" in _STATE:
        return _STATE["runner"]

    import jax
    import jax.numpy as jnp
    from jax.sharding import Mesh, PartitionSpec
    from jax.experimental.shard_map import shard_map
    from concourse import mybir
    from concourse.bass2jax import (
        _bass_exec_p,
        install_neuronx_cc_hook,
        partition_id_tensor,
    )

    install_neuronx_cc_hook()
    nc = _get_nc()
    assert nc.dbg_addr is None
    partition_name = nc.partition_id_tensor.name if nc.partition_id_tensor else None

    in_names: list[str] = []
    out_names: list[str] = []
    out_avals: list = []
    for alloc in nc.m.functions[0].allocations:
        if not isinstance(alloc, mybir.MemoryLocationSet):
            continue
        name = alloc.memorylocations[0].name
        if alloc.kind == "ExternalInput":
            if name != partition_name:
                in_names.append(name)
        elif alloc.kind == "ExternalOutput":
            shape = tuple(alloc.tensor_shape)
            dtype = mybir.dt.np(alloc.dtype)
            out_names.append(name)
            out_avals.append(jax.core.ShapedArray(shape, dtype))
    n_params = len(in_names)
    all_names = in_names + out_names
    if partition_name is not None:
        all_names = all_names + [partition_name]

    def _body(*args):
        operands = list(args)
        if partition_name is not None:
            operands.append(partition_id_tensor())
        outs = _bass_exec_p.bind(
            *operands,
            out_avals=tuple(out_avals),
            in_names=tuple(all_names),
            out_names=tuple(out_names),
            lowering_input_output_aliases=(),
            sim_require_finite=True,
            sim_require_nnan=True,
            nc=nc,
        )
        return tuple(outs)

    devices = jax.devices()[:N_CORES]
    mesh = Mesh(np.asarray(devices), ("core",))
    n_outs = len(out_names)
    in_specs = (PartitionSpec("core"),) * (n_params + n_outs)
    out_specs = (PartitionSpec("core"),) * n_outs
    sharded = jax.jit(
        shard_map(
            _body, mesh=mesh, in_specs=in_specs, out_specs=out_specs, check_rep=False
        ),
        keep_unused=True,
    )

    class _Runner:
        def prepare(self, in_maps: list[dict]) -> list:
            """Stage concatenated inputs + zero outputs once for repeat calls."""
            concat_in = [
                np.concatenate([np.asarray(m[name]) for m in in_maps], axis=0)
                for name in in_names
            ]
            concat_zeros = [
                np.zeros((N_CORES * a.shape[0], *a.shape[1:]), a.dtype)
                for a in out_avals
            ]
            return concat_in + concat_zeros

        def run_prepared(self, args: list):
            out_arrs = sharded(*args)
            jax.block_until_ready(out_arrs)
            return out_arrs

        def __call__(self, in_maps: list[dict]) -> list[dict]:
            out_arrs = self.run_prepared(self.prepare(in_maps))
            return [
                {
                    name: np.asarray(out_arrs[i]).reshape(
                        N_CORES, *out_avals[i].shape
                    )[c]
                    for i, name in enumerate(out_names)
                }
                for c in range(N_CORES)
            ]

    runner = _Runner()
    _STATE["runner"] = runner
    return runner


def run_device(in_maps: list[dict]) -> list[dict]:
    """Compile (cached) + execute the SPMD program on cores 0..7."""
    return _get_runner()(in_maps)


def kernel(preds: np.ndarray, gts: np.ndarray) -> np.ndarray:
    in_maps = _prep_in_maps(preds, gts)
    results = run_device(in_maps)
    return _finish(results)
